# revision 8
# baseline (speedup 1.0000x reference)
"""DD-RoPE kernel for 8x TRN2 NeuronCores (v5).

Reference computation (B=4, T=4096, D=2048, P=256):
    deltas = einsum('btd,pd->btp', x, W) + b     # (B, T, P)
    angles = cumsum(deltas, axis=1)
    out = concat([x1*cos(a) - x2*sin(a), x2*cos(a) + x1*sin(a), x[..., 512:]], -1)

Sharding: 8 shards = 4 batches x 2 T-halves (2048 each), data-parallel.
The cumsum is handled with host-computed fp64 "block bases": the exact
cumulative angle (using the TRUE fp32 W, in turns, reduced mod 1) at every
512-step boundary.  Each on-device prefix scan then only spans <= 512 steps,
so the fp16-weight quantization drift is limited to sqrt(512) steps instead
of sqrt(T), which keeps a single fp16 matmul pass inside the error budget.

This problem sits at the DMA/PE ridge: 9.4 MB input + 2.1 MB output per core
(~30 us at the ~350 GB/s core DMA ceiling) vs ~31 us of fp16 matmul. The
schedule is built around streaming:
  - sync HW queue carries x0a, x0b, x1..x3 (need order, FIFO completion),
    then the four output DMAs (program order keeps their completion waits
    from ever blocking an x prefetch)
  - scalar HW queue carries bs, bc, wh0, wh1 in parallel: the first matmul
    group is gated on just bs+bc+wh0 (0.5 MB) + x0a (1 MB)
  - per (tb, h): 16 chunk matmuls -> one 512 scan (bias via scan data1,
    initial = host base) -> magic-round range reduction -> Sin activations
    (cos via sin(pi/2 - 2pi|r|)) -> fp16 rotation split DVE/GpSimd
  - the last time block runs its scan/trig/rotation at 256 granularity with
    rotation all on DVE to shorten the end-of-kernel drain chain
"""

import sys

if "/opt/trn_rl_repo" not in sys.path:
    sys.path.insert(0, "/opt/trn_rl_repo")

from contextlib import ExitStack

import numpy as np

import concourse.bacc as bacc
import concourse.bass as bass
import concourse.mybir as mybir
import concourse.tile as tile
from concourse.bass_utils import run_bass_kernel_spmd

F32 = mybir.dt.float32
F16 = mybir.dt.float16
ADD = mybir.AluOpType.add
SUB = mybir.AluOpType.subtract
IDENT = mybir.ActivationFunctionType.Identity
ABS = mybir.ActivationFunctionType.Abs
SIN = mybir.ActivationFunctionType.Sin

D = 2048          # input feature dim (contraction)
P = 256           # delta-pairs dim
ROT = 2 * P       # rotated columns (512)
TL = 2048         # time steps per shard
TB = 512          # time block (one PSUM bank at fp32) == scan block
NT = TL // TB     # time blocks per shard (4)
KC = D // 128     # contraction chunks (16)
NB = NT + 1       # base columns per half (512-grid plus the 1792 boundary)
N_CORES = 8

MAGIC = 12582912.0          # 1.5 * 2**23: fp32 round-to-int magic constant
SCALE_2PI = 6.28310         # slightly < 2*pi so Sin args stay inside [-pi, pi]
COS_BIAS = 1.5707964        # ~pi/2 (fp32)


def build_program(tl: int = TL) -> bass.Bass:
    nt = tl // TB
    nc = bacc.Bacc("TRN2", target_bir_lowering=False, debug=False)

    # Host-pre-tiled inputs: every DMA below reads one dense DRAM block.
    # xf row block tb: [128, KC*TB] fp16 (d-chunks along the free dim)
    xf = nc.dram_tensor("xf", [nt * 128, KC * TB], F16,
                        kind="ExternalInput").ap()
    # per-pair-half weights: [128, KC*128] fp16, d-chunks along free
    wh0 = nc.dram_tensor("wh0", [128, KC * 128], F16,
                         kind="ExternalInput").ap()
    wh1 = nc.dram_tensor("wh1", [128, KC * 128], F16,
                         kind="ExternalInput").ap()
    # bias per pair-half (turns), [128, 2] fp32: col h = b[h*128:(h+1)*128]
    bc = nc.dram_tensor("bc", [128, 2], F32, kind="ExternalInput").ap()
    # block angle bases (turns, mod 1), [128, 2*NB] fp32: col h*NB + i =
    # base at local t = [0, 512, 1024, 1536, 1792][i] for pair-half h
    bs = nc.dram_tensor("bs", [128, 2 * NB], F32, kind="ExternalInput").ap()
    # out row block tb: [128, 4*TB] fp16 (quadrants o1h0|o1h1|o2h0|o2h1)
    outT = nc.dram_tensor("outT", [nt * 128, 4 * TB], F16,
                          kind="ExternalOutput").ap()

    with tile.TileContext(nc) as tc, ExitStack() as ctx:
        const_pool = ctx.enter_context(tc.tile_pool(name="const", bufs=1))
        w_pool = ctx.enter_context(tc.tile_pool(name="w", bufs=1))
        x_pool = ctx.enter_context(tc.tile_pool(name="x", bufs=4))
        psum_pool = ctx.enter_context(tc.tile_pool(name="psum", bufs=3, space="PSUM"))
        ang_pool = ctx.enter_context(tc.tile_pool(name="ang", bufs=3, space="PSUM"))
        trig_pool = ctx.enter_context(tc.tile_pool(name="trig", bufs=2))
        rot_pool = ctx.enter_context(tc.tile_pool(name="rot", bufs=2))
        out_pool = ctx.enter_context(tc.tile_pool(name="out", bufs=2))

        # x prefetch on the sync HW queue in need order; x0 in halves so the
        # first matmul group is gated on 1 MB, not 2.
        x_tiles = []
        for tb in range(nt):
            xall = x_pool.tile([128, KC * TB], F16, tag="xall")
            x_tiles.append(xall)
        half = KC * TB // 2
        nc.sync.dma_start(x_tiles[0][:, 0:half],
                          xf[0:128, 0:half])
        nc.sync.dma_start(x_tiles[0][:, half:2 * half],
                          xf[0:128, half:2 * half])
        for tb in range(1, nt):
            nc.sync.dma_start(x_tiles[tb][:],
                              xf[tb * 128:(tb + 1) * 128, :])

        # consts + weights on the scalar HW queue (parallel ring): tiny bs/bc
        # first, then the two weight halves in use order.
        bs_sb = const_pool.tile([128, 2 * NB], F32, tag="bs")
        nc.scalar.dma_start(bs_sb[:], bs[:])
        bc_sb = const_pool.tile([128, 2], F32, tag="bc")
        nc.scalar.dma_start(bc_sb[:], bc[:])
        w_sb = []
        for h in range(2):
            whs = w_pool.tile([128, KC * 128], F16, tag=f"wh{h}")
            nc.scalar.dma_start(whs[:], (wh0 if h == 0 else wh1)[:])
            w_sb.append(whs)

        magic_sb = const_pool.tile([128, 1], F32, tag="magic")
        nc.gpsimd.memset(magic_sb[:], MAGIC)
        cosb_sb = const_pool.tile([128, 1], F32, tag="cosb")
        nc.gpsimd.memset(cosb_sb[:], COS_BIAS)
        # bias broadcast tile for the scan's data1: [128, 2*TB], col block h
        # filled with b[h*128+p] (memset 0 then add the per-partition bias)
        bt_sb = const_pool.tile([128, 2 * TB], F32, tag="bt")
        nc.gpsimd.memset(bt_sb[:], 0.0)
        for h in range(2):
            nc.scalar.activation(bt_sb[:, h * TB:(h + 1) * TB],
                                 bt_sb[:, h * TB:(h + 1) * TB], IDENT,
                                 bias=bc_sb[:, h:h + 1])

        for tb in range(nt):
            xall = x_tiles[tb]
            oall = out_pool.tile([128, 4 * TB], F16, tag="oall")
            last = tb == nt - 1
            # the final block runs its post-matmul chain at 256 granularity
            # (and rotation all on DVE) to shorten the kernel drain
            splits = 2 if last else 1
            sw = TB // splits

            for h in range(2):
                # deltas^T in PSUM: single fp16 pass over 16 d-chunks
                dp = psum_pool.tile([128, TB], F32, tag="dp")
                for d in range(KC):
                    ws = slice(d * 128, (d + 1) * 128)
                    xs = slice(d * TB, (d + 1) * TB)
                    nc.tensor.matmul(dp[:], w_sb[h][:, ws], xall[:, xs],
                                     start=(d == 0), stop=(d == KC - 1))

                ang = ang_pool.tile([128, TB], F32, tag="ang")
                a_s = trig_pool.tile([128, TB], F32, tag="a_s")
                rs = trig_pool.tile([128, TB], F32, tag="rs")
                sin_t = trig_pool.tile([128, TB], F16, tag="sin")
                ab = trig_pool.tile([128, TB], F32, tag="ab")
                cos_t = trig_pool.tile([128, TB], F16, tag="cos")
                t1 = rot_pool.tile([128, TB], F16, tag="t1")
                t2 = rot_pool.tile([128, TB], F16, tag="t2")
                t3 = rot_pool.tile([128, TB], F16, tag="t3")
                t4 = rot_pool.tile([128, TB], F16, tag="t4")

                for s in range(splits):
                    c = slice(s * sw, (s + 1) * sw)
                    # base column: local t = tb*512 (+256 for the tail split)
                    bi = h * NB + (tb if s == 0 else NB - 1)
                    # cumulative angle (turns): prefix scan, initial = host
                    # base, +b per step via data1
                    nc.vector.tensor_tensor_scan(
                        ang[:, c], dp[:, c], bt_sb[:, h * TB + s * sw:
                                                   h * TB + (s + 1) * sw],
                        initial=bs_sb[:, bi:bi + 1],
                        op0=ADD, op1=ADD)

                    # range reduction (turns): rs = y - round(y) in [-.5, .5]
                    nc.scalar.activation(a_s[:, c], ang[:, c], IDENT,
                                         bias=magic_sb[:], scale=-1.0)
                    nc.vector.scalar_tensor_tensor(rs[:, c], a_s[:, c],
                                                   MAGIC, ang[:, c],
                                                   op0=SUB, op1=ADD)
                    nc.scalar.activation(sin_t[:, c], rs[:, c], SIN,
                                         scale=SCALE_2PI)
                    # cos(2pi r) = sin(pi/2 - 2pi|r|), |r| <= 0.5
                    nc.scalar.activation(ab[:, c], rs[:, c], ABS)
                    nc.scalar.activation(cos_t[:, c], ab[:, c], SIN,
                                         scale=-SCALE_2PI, bias=cosb_sb[:])

                    # rotation: x1^T = d-chunk h, x2^T = d-chunk 2+h of xall
                    x1s = xall[:, h * TB + s * sw:h * TB + (s + 1) * sw]
                    x2s = xall[:, (2 + h) * TB + s * sw:
                               (2 + h) * TB + (s + 1) * sw]
                    o1 = oall[:, h * TB + s * sw:h * TB + (s + 1) * sw]
                    o2 = oall[:, (2 + h) * TB + s * sw:
                              (2 + h) * TB + (s + 1) * sw]
                    nc.vector.tensor_mul(t1[:, c], x1s, cos_t[:, c])
                    nc.vector.tensor_mul(t2[:, c], x2s, sin_t[:, c])
                    nc.vector.tensor_sub(o1, t1[:, c], t2[:, c])
                    g = nc.vector if last else nc.gpsimd
                    g.tensor_mul(t3[:, c], x2s, cos_t[:, c])
                    g.tensor_mul(t4[:, c], x1s, sin_t[:, c])
                    g.tensor_add(o2, t3[:, c], t4[:, c])

            # out DMA on the sync queue: all x prefetches are already issued,
            # so its completion wait never blocks them; the ring is idle by
            # the time outputs start flowing.
            nc.sync.dma_start(outT[tb * 128:(tb + 1) * 128, :], oall[:])

    nc.compile()
    return nc


_NC_CACHE: dict = {}


def _get_nc():
    if "nc" not in _NC_CACHE:
        _NC_CACHE["nc"] = build_program()
    return _NC_CACHE["nc"]


def _tile_x(xt16: np.ndarray, nt: int) -> np.ndarray:
    """[D, tl] fp16 -> [nt*128, KC*TB]: row block tb, d-chunks along free."""
    tl = xt16.shape[1]
    a = xt16.reshape(KC, 128, tl // TB, TB).transpose(2, 1, 0, 3)
    return np.ascontiguousarray(a.reshape((tl // TB) * 128, KC * TB))


def prepare_weights(W: np.ndarray, b: np.ndarray):
    inv2pi = 1.0 / (2.0 * np.pi)
    Wt = W.astype(np.float64).T * inv2pi                           # [D, P]
    bt = b.astype(np.float64) * inv2pi                             # [P]
    whf = Wt.astype(np.float16)
    # [D, P] -> per half h: [128, KC*128] with d-chunks along free dim
    w4 = whf.reshape(KC, 128, 2, 128)                              # d-chunk, dd, h, pp
    wh_ins = [np.ascontiguousarray(
        w4[:, :, h, :].transpose(1, 0, 2).reshape(128, KC * 128))
        for h in range(2)]
    bc_in = np.ascontiguousarray(
        bt.astype(np.float32).reshape(2, 128).T)                   # [128, 2]
    return wh_ins, bc_in, Wt, bt


def make_in_maps(x: np.ndarray, W: np.ndarray, b: np.ndarray):
    B = x.shape[0]
    wh_ins, bc_in, Wt, bt = prepare_weights(W, b)

    # fp64 cumulative angle at every 256-step boundary, per batch (in turns,
    # using the TRUE W so on-device fp16-weight drift spans <= 512 steps):
    # one pass of 256-block sums over x, then a small [16, D] @ [D, P] matmul
    T = x.shape[1]
    SG = 256
    nblk = T // SG                                                  # 16
    xblk = x.reshape(B, nblk, SG, D).sum(axis=2, dtype=np.float64)  # [B,16,D]
    dblk = xblk @ Wt + SG * bt                                      # [B,16,P]
    bases = np.zeros((B, nblk, P))
    np.cumsum(dblk[:, :-1], axis=1, out=bases[:, 1:])               # exclusive
    bases -= np.round(bases)                                        # mod 1

    in_maps = []
    for c in range(N_CORES):
        bb, hh = c // 2, c % 2
        xt16 = x[bb, hh * TL:(hh + 1) * TL, :].T.astype(np.float16)
        # base columns at local t = [0, 512, 1024, 1536, 1792]
        gsel = [hh * 8 + g for g in (0, 2, 4, 6, 7)]
        bs_in = np.empty((128, 2 * NB), np.float32)
        for h in range(2):
            for i, g in enumerate(gsel):
                bs_in[:, h * NB + i] = bases[bb, g, h * 128:(h + 1) * 128]
        in_maps.append({
            "xf": _tile_x(xt16, NT),
            "wh0": wh_ins[0],
            "wh1": wh_ins[1],
            "bc": bc_in,
            "bs": bs_in,
        })
    return in_maps


def assemble_output(x: np.ndarray, results) -> np.ndarray:
    B, T, Din = x.shape
    out = np.empty((B, T, Din), np.float32)
    out[:, :, ROT:] = x[:, :, ROT:]
    for c in range(N_CORES):
        bb, hh = c // 2, c % 2
        r = results[c]["outT"].reshape(NT, 128, 4, TB)
        # [tb, pp, q(oi,h), u] -> [t_local(tb,u), p(oi,h,pp)]
        blk = r.transpose(0, 3, 2, 1).reshape(TL, ROT)
        out[bb, hh * TL:(hh + 1) * TL, :ROT] = blk
    return out


def kernel(x: np.ndarray, W: np.ndarray, b: np.ndarray) -> np.ndarray:
    nc = _get_nc()
    in_maps = make_in_maps(x, W, b)
    res = run_bass_kernel_spmd(nc, in_maps, list(range(N_CORES)))
    return assemble_output(x, res.results)


# revision 12
# speedup vs baseline: 1.2042x; 1.2042x over previous
"""DD-RoPE kernel for 8x TRN2 NeuronCores (v5).

Reference computation (B=4, T=4096, D=2048, P=256):
    deltas = einsum('btd,pd->btp', x, W) + b     # (B, T, P)
    angles = cumsum(deltas, axis=1)
    out = concat([x1*cos(a) - x2*sin(a), x2*cos(a) + x1*sin(a), x[..., 512:]], -1)

Sharding: 8 shards = 4 batches x 2 T-halves (2048 each), data-parallel.
The cumsum is handled with host-computed fp64 "block bases": the exact
cumulative angle (using the TRUE fp32 W, in turns, reduced mod 1) at every
512-step boundary.  Each on-device prefix scan then only spans <= 512 steps,
so the fp16-weight quantization drift is limited to sqrt(512) steps instead
of sqrt(T), which keeps a single fp16 matmul pass inside the error budget.

This problem sits at the DMA/PE ridge: 9.4 MB input + 2.1 MB output per core
(~30 us at the ~350 GB/s core DMA ceiling) vs ~31 us of fp16 matmul. The
schedule is built around streaming:
  - sync HW queue carries x0a, x0b, x1..x3 (need order, FIFO completion),
    then the four output DMAs (program order keeps their completion waits
    from ever blocking an x prefetch)
  - scalar HW queue carries bs, bc, wh0, wh1 in parallel: the first matmul
    group is gated on just bs+bc+wh0 (0.5 MB) + x0a (1 MB)
  - per (tb, h): 16 chunk matmuls -> one 512 scan (bias via scan data1,
    initial = host base) -> magic-round range reduction -> Sin activations
    (cos via sin(pi/2 - 2pi|r|)) -> fp16 rotation split DVE/GpSimd
  - the last time block runs its scan/trig/rotation at 256 granularity with
    rotation all on DVE to shorten the end-of-kernel drain chain
"""

import sys

if "/opt/trn_rl_repo" not in sys.path:
    sys.path.insert(0, "/opt/trn_rl_repo")

from contextlib import ExitStack

import numpy as np

import concourse.bacc as bacc
import concourse.bass as bass
import concourse.mybir as mybir
import concourse.tile as tile
from concourse.bass_utils import run_bass_kernel_spmd

F32 = mybir.dt.float32
F16 = mybir.dt.float16
ADD = mybir.AluOpType.add
SUB = mybir.AluOpType.subtract
IDENT = mybir.ActivationFunctionType.Identity
ABS = mybir.ActivationFunctionType.Abs
SIN = mybir.ActivationFunctionType.Sin

D = 2048          # input feature dim (contraction)
P = 256           # delta-pairs dim
ROT = 2 * P       # rotated columns (512)
TL = 2048         # time steps per shard
TB = 512          # time block (one PSUM bank at fp32) == scan block
NT = TL // TB     # time blocks per shard (4)
KC = D // 128     # contraction chunks (16)
NB = NT + 1       # base columns per half (512-grid plus the 1792 boundary)
N_CORES = 8

MAGIC = 12582912.0          # 1.5 * 2**23: fp32 round-to-int magic constant
SCALE_2PI = 6.28310         # slightly < 2*pi so Sin args stay inside [-pi, pi]
COS_BIAS = 1.5707964        # ~pi/2 (fp32)


def build_program(tl: int = TL) -> bass.Bass:
    nt = tl // TB
    nc = bacc.Bacc("TRN2", target_bir_lowering=False, debug=False)

    # Host-pre-tiled inputs: every DMA below reads one dense DRAM block.
    # xf row block tb: [128, KC*TB] fp16 (d-chunks along the free dim)
    xf = nc.dram_tensor("xf", [nt * 128, KC * TB], F16,
                        kind="ExternalInput").ap()
    # per-pair-half weights: [128, KC*128] fp16, d-chunks along free
    wh0 = nc.dram_tensor("wh0", [128, KC * 128], F16,
                         kind="ExternalInput").ap()
    wh1 = nc.dram_tensor("wh1", [128, KC * 128], F16,
                         kind="ExternalInput").ap()
    # bias per pair-half (turns), [128, 2] fp32: col h = b[h*128:(h+1)*128]
    bc = nc.dram_tensor("bc", [128, 2], F32, kind="ExternalInput").ap()
    # block angle bases (turns, mod 1), [128, 2*NB] fp32: col h*NB + i =
    # base at local t = [0, 512, 1024, 1536, 1792][i] for pair-half h
    bs = nc.dram_tensor("bs", [128, 2 * NB], F32, kind="ExternalInput").ap()
    # out row block tb: [128, 4*TB] fp16 (quadrants o1h0|o1h1|o2h0|o2h1)
    outT = nc.dram_tensor("outT", [nt * 128, 4 * TB], F16,
                          kind="ExternalOutput").ap()

    with tile.TileContext(nc) as tc, ExitStack() as ctx:
        const_pool = ctx.enter_context(tc.tile_pool(name="const", bufs=1))
        w_pool = ctx.enter_context(tc.tile_pool(name="w", bufs=1))
        x_pool = ctx.enter_context(tc.tile_pool(name="x", bufs=4))
        psum_pool = ctx.enter_context(tc.tile_pool(name="psum", bufs=3, space="PSUM"))
        ang_pool = ctx.enter_context(tc.tile_pool(name="ang", bufs=3, space="PSUM"))
        trig_pool = ctx.enter_context(tc.tile_pool(name="trig", bufs=2))
        rot_pool = ctx.enter_context(tc.tile_pool(name="rot", bufs=2))
        out_pool = ctx.enter_context(tc.tile_pool(name="out", bufs=2))

        # The sync HW ring is the only fast DMA path (~380 GB/s; the scalar
        # ring gets starved to <100 GB/s when sync is active). Stream every
        # large tensor on it in exact need order, x blocks split in halves
        # (separate tiles, so the first matmul group is gated on 1 MB only):
        #   x0a, wh0, x0b, wh1, x1a, x1b, x2a, x2b, x3a, x3b, out0..out3
        half = KC * TB // 2
        x_tiles = []
        for tb in range(nt):
            xa = x_pool.tile([128, half], F16, tag="xa")
            xb = x_pool.tile([128, half], F16, tag="xb")
            x_tiles.append((xa, xb))
        w_sb = []
        for h in range(2):
            whs = w_pool.tile([128, KC * 128], F16, tag=f"wh{h}")
            w_sb.append(whs)
        nc.sync.dma_start(x_tiles[0][0][:], xf[0:128, 0:half])
        nc.sync.dma_start(w_sb[0][:], wh0[:])
        nc.sync.dma_start(x_tiles[0][1][:], xf[0:128, half:2 * half])
        nc.sync.dma_start(w_sb[1][:], wh1[:])
        for tb in range(1, nt):
            nc.sync.dma_start(x_tiles[tb][0][:],
                              xf[tb * 128:(tb + 1) * 128, 0:half])
            nc.sync.dma_start(x_tiles[tb][1][:],
                              xf[tb * 128:(tb + 1) * 128, half:2 * half])

        # tiny consts on the scalar ring (needed by the first scan at ~16us;
        # even the starved ring lands them in time)
        bs_sb = const_pool.tile([128, 2 * NB], F32, tag="bs")
        nc.scalar.dma_start(bs_sb[:], bs[:])
        bc_sb = const_pool.tile([128, 2], F32, tag="bc")
        nc.scalar.dma_start(bc_sb[:], bc[:])

        magic_sb = const_pool.tile([128, 1], F32, tag="magic")
        nc.gpsimd.memset(magic_sb[:], MAGIC)
        cosb_sb = const_pool.tile([128, 1], F32, tag="cosb")
        nc.gpsimd.memset(cosb_sb[:], COS_BIAS)
        # bias broadcast tile for the scan's data1: [128, 2*TB], col block h
        # filled with b[h*128+p] (memset 0 then add the per-partition bias)
        bt_sb = const_pool.tile([128, 2 * TB], F32, tag="bt")
        nc.gpsimd.memset(bt_sb[:], 0.0)
        for h in range(2):
            nc.scalar.activation(bt_sb[:, h * TB:(h + 1) * TB],
                                 bt_sb[:, h * TB:(h + 1) * TB], IDENT,
                                 bias=bc_sb[:, h:h + 1])

        for tb in range(nt):
            xa, xb = x_tiles[tb]
            oall = out_pool.tile([128, 4 * TB], F16, tag="oall")
            last = tb == nt - 1
            # the final block runs its post-matmul chain at 256 granularity
            # (and rotation all on DVE) to shorten the kernel drain
            splits = 2 if last else 1
            sw = TB // splits

            for h in range(2):
                # deltas^T in PSUM: single fp16 pass over 16 d-chunks
                dp = psum_pool.tile([128, TB], F32, tag="dp")
                for d in range(KC):
                    ws = slice(d * 128, (d + 1) * 128)
                    xt = xa if d < KC // 2 else xb
                    xs = slice((d % (KC // 2)) * TB,
                               (d % (KC // 2) + 1) * TB)
                    nc.tensor.matmul(dp[:], w_sb[h][:, ws], xt[:, xs],
                                     start=(d == 0), stop=(d == KC - 1))

                ang = ang_pool.tile([128, TB], F32, tag="ang")
                a_s = trig_pool.tile([128, TB], F32, tag="a_s")
                rs = trig_pool.tile([128, TB], F32, tag="rs")
                sin_t = trig_pool.tile([128, TB], F16, tag="sin")
                ab = trig_pool.tile([128, TB], F32, tag="ab")
                cos_t = trig_pool.tile([128, TB], F16, tag="cos")
                t1 = rot_pool.tile([128, TB], F16, tag="t1")
                t2 = rot_pool.tile([128, TB], F16, tag="t2")
                t3 = rot_pool.tile([128, TB], F16, tag="t3")
                t4 = rot_pool.tile([128, TB], F16, tag="t4")

                for s in range(splits):
                    c = slice(s * sw, (s + 1) * sw)
                    # base column: local t = tb*512 (+256 for the tail split)
                    bi = h * NB + (tb if s == 0 else NB - 1)
                    # cumulative angle (turns): prefix scan, initial = host
                    # base, +b per step via data1
                    nc.vector.tensor_tensor_scan(
                        ang[:, c], dp[:, c], bt_sb[:, h * TB + s * sw:
                                                   h * TB + (s + 1) * sw],
                        initial=bs_sb[:, bi:bi + 1],
                        op0=ADD, op1=ADD)

                    # range reduction (turns): rs = y - round(y) in [-.5, .5]
                    nc.scalar.activation(a_s[:, c], ang[:, c], IDENT,
                                         bias=magic_sb[:], scale=-1.0)
                    nc.vector.scalar_tensor_tensor(rs[:, c], a_s[:, c],
                                                   MAGIC, ang[:, c],
                                                   op0=SUB, op1=ADD)
                    nc.scalar.activation(sin_t[:, c], rs[:, c], SIN,
                                         scale=SCALE_2PI)
                    # cos(2pi r) = sin(pi/2 - 2pi|r|), |r| <= 0.5
                    nc.scalar.activation(ab[:, c], rs[:, c], ABS)
                    nc.scalar.activation(cos_t[:, c], ab[:, c], SIN,
                                         scale=-SCALE_2PI, bias=cosb_sb[:])

                    # rotation: x1^T = d-chunk h, x2^T = d-chunk 2+h (in xa)
                    x1s = xa[:, h * TB + s * sw:h * TB + (s + 1) * sw]
                    x2s = xa[:, (2 + h) * TB + s * sw:
                             (2 + h) * TB + (s + 1) * sw]
                    o1 = oall[:, h * TB + s * sw:h * TB + (s + 1) * sw]
                    o2 = oall[:, (2 + h) * TB + s * sw:
                              (2 + h) * TB + (s + 1) * sw]
                    nc.vector.tensor_mul(t1[:, c], x1s, cos_t[:, c])
                    nc.vector.tensor_mul(t2[:, c], x2s, sin_t[:, c])
                    nc.vector.tensor_sub(o1, t1[:, c], t2[:, c])
                    g = nc.vector if last else nc.gpsimd
                    g.tensor_mul(t3[:, c], x2s, cos_t[:, c])
                    g.tensor_mul(t4[:, c], x1s, sin_t[:, c])
                    g.tensor_add(o2, t3[:, c], t4[:, c])

            # out DMA on the sync queue: all x prefetches are already issued,
            # so its completion wait never blocks them; the ring is idle by
            # the time outputs start flowing.
            nc.sync.dma_start(outT[tb * 128:(tb + 1) * 128, :], oall[:])

    nc.compile()
    return nc


_NC_CACHE: dict = {}


def _get_nc():
    if "nc" not in _NC_CACHE:
        _NC_CACHE["nc"] = build_program()
    return _NC_CACHE["nc"]


def _tile_x(xt16: np.ndarray, nt: int) -> np.ndarray:
    """[D, tl] fp16 -> [nt*128, KC*TB]: row block tb, d-chunks along free."""
    tl = xt16.shape[1]
    a = xt16.reshape(KC, 128, tl // TB, TB).transpose(2, 1, 0, 3)
    return np.ascontiguousarray(a.reshape((tl // TB) * 128, KC * TB))


def prepare_weights(W: np.ndarray, b: np.ndarray):
    inv2pi = 1.0 / (2.0 * np.pi)
    Wt = W.astype(np.float64).T * inv2pi                           # [D, P]
    bt = b.astype(np.float64) * inv2pi                             # [P]
    whf = Wt.astype(np.float16)
    # [D, P] -> per half h: [128, KC*128] with d-chunks along free dim
    w4 = whf.reshape(KC, 128, 2, 128)                              # d-chunk, dd, h, pp
    wh_ins = [np.ascontiguousarray(
        w4[:, :, h, :].transpose(1, 0, 2).reshape(128, KC * 128))
        for h in range(2)]
    bc_in = np.ascontiguousarray(
        bt.astype(np.float32).reshape(2, 128).T)                   # [128, 2]
    return wh_ins, bc_in, Wt, bt


def make_in_maps(x: np.ndarray, W: np.ndarray, b: np.ndarray):
    B = x.shape[0]
    wh_ins, bc_in, Wt, bt = prepare_weights(W, b)

    # fp64 cumulative angle at every 256-step boundary, per batch (in turns,
    # using the TRUE W so on-device fp16-weight drift spans <= 512 steps):
    # one pass of 256-block sums over x, then a small [16, D] @ [D, P] matmul
    T = x.shape[1]
    SG = 256
    nblk = T // SG                                                  # 16
    xblk = x.reshape(B, nblk, SG, D).sum(axis=2, dtype=np.float64)  # [B,16,D]
    dblk = xblk @ Wt + SG * bt                                      # [B,16,P]
    bases = np.zeros((B, nblk, P))
    np.cumsum(dblk[:, :-1], axis=1, out=bases[:, 1:])               # exclusive
    bases -= np.round(bases)                                        # mod 1

    in_maps = []
    for c in range(N_CORES):
        bb, hh = c // 2, c % 2
        xt16 = x[bb, hh * TL:(hh + 1) * TL, :].T.astype(np.float16)
        # base columns at local t = [0, 512, 1024, 1536, 1792]
        gsel = [hh * 8 + g for g in (0, 2, 4, 6, 7)]
        bs_in = np.empty((128, 2 * NB), np.float32)
        for h in range(2):
            for i, g in enumerate(gsel):
                bs_in[:, h * NB + i] = bases[bb, g, h * 128:(h + 1) * 128]
        in_maps.append({
            "xf": _tile_x(xt16, NT),
            "wh0": wh_ins[0],
            "wh1": wh_ins[1],
            "bc": bc_in,
            "bs": bs_in,
        })
    return in_maps


def assemble_output(x: np.ndarray, results) -> np.ndarray:
    B, T, Din = x.shape
    out = np.empty((B, T, Din), np.float32)
    out[:, :, ROT:] = x[:, :, ROT:]
    for c in range(N_CORES):
        bb, hh = c // 2, c % 2
        r = results[c]["outT"].reshape(NT, 128, 4, TB)
        # [tb, pp, q(oi,h), u] -> [t_local(tb,u), p(oi,h,pp)]
        blk = r.transpose(0, 3, 2, 1).reshape(TL, ROT)
        out[bb, hh * TL:(hh + 1) * TL, :ROT] = blk
    return out


def kernel(x: np.ndarray, W: np.ndarray, b: np.ndarray) -> np.ndarray:
    nc = _get_nc()
    in_maps = make_in_maps(x, W, b)
    res = run_bass_kernel_spmd(nc, in_maps, list(range(N_CORES)))
    return assemble_output(x, res.results)


# revision 19
# speedup vs baseline: 1.2453x; 1.0341x over previous
"""DD-RoPE kernel for 8x TRN2 NeuronCores (v5).

Reference computation (B=4, T=4096, D=2048, P=256):
    deltas = einsum('btd,pd->btp', x, W) + b     # (B, T, P)
    angles = cumsum(deltas, axis=1)
    out = concat([x1*cos(a) - x2*sin(a), x2*cos(a) + x1*sin(a), x[..., 512:]], -1)

Sharding: 8 shards = 4 batches x 2 T-halves (2048 each), data-parallel.
The cumsum is handled with host-computed fp64 "block bases": the exact
cumulative angle (using the TRUE fp32 W, in turns, reduced mod 1) at every
512-step boundary.  Each on-device prefix scan then only spans <= 512 steps,
so the fp16-weight quantization drift is limited to sqrt(512) steps instead
of sqrt(T), which keeps a single fp16 matmul pass inside the error budget.

This problem sits at the DMA/PE ridge: 9.4 MB input + 2.1 MB output per core
(~30 us at the ~350 GB/s core DMA ceiling) vs ~31 us of fp16 matmul. The
schedule is built around streaming:
  - sync HW queue carries x0a, x0b, x1..x3 (need order, FIFO completion),
    then the four output DMAs (program order keeps their completion waits
    from ever blocking an x prefetch)
  - scalar HW queue carries bs, bc, wh0, wh1 in parallel: the first matmul
    group is gated on just bs+bc+wh0 (0.5 MB) + x0a (1 MB)
  - per (tb, h): 16 chunk matmuls -> one 512 scan (bias via scan data1,
    initial = host base) -> magic-round range reduction -> Sin activations
    (cos via sin(pi/2 - 2pi|r|)) -> fp16 rotation split DVE/GpSimd
  - the last time block runs its scan/trig/rotation at 256 granularity with
    rotation all on DVE to shorten the end-of-kernel drain chain
"""

import sys

if "/opt/trn_rl_repo" not in sys.path:
    sys.path.insert(0, "/opt/trn_rl_repo")

from contextlib import ExitStack

import numpy as np

import concourse.bacc as bacc
import concourse.bass as bass
import concourse.mybir as mybir
import concourse.tile as tile
from concourse.bass_utils import run_bass_kernel_spmd

F32 = mybir.dt.float32
F16 = mybir.dt.float16
ADD = mybir.AluOpType.add
SUB = mybir.AluOpType.subtract
IDENT = mybir.ActivationFunctionType.Identity
ABS = mybir.ActivationFunctionType.Abs
SIN = mybir.ActivationFunctionType.Sin

D = 2048          # input feature dim (contraction)
P = 256           # delta-pairs dim
ROT = 2 * P       # rotated columns (512)
TL = 2048         # time steps per shard
TB = 512          # time block (one PSUM bank at fp32) == scan block
NT = TL // TB     # time blocks per shard (4)
KC = D // 128     # contraction chunks (16)
NB = NT + 1       # base columns per half (512-grid plus the 1792 boundary)
N_CORES = 8

MAGIC = 12582912.0          # 1.5 * 2**23: fp32 round-to-int magic constant
SCALE_2PI = 6.28310         # slightly < 2*pi so Sin args stay inside [-pi, pi]
COS_BIAS = 1.5707964        # ~pi/2 (fp32)


def build_program(tl: int = TL) -> bass.Bass:
    nt = tl // TB
    nc = bacc.Bacc("TRN2", target_bir_lowering=False, debug=False)

    # Host-pre-tiled inputs: every DMA below reads one dense DRAM block.
    # xf row block tb: [128, KC*TB] fp16 (d-chunks along the free dim)
    xf = nc.dram_tensor("xf", [nt * 128, KC * TB], F16,
                        kind="ExternalInput").ap()
    # per-pair-half weights: [128, KC*128] fp16, d-chunks along free
    wh0 = nc.dram_tensor("wh0", [128, KC * 128], F16,
                         kind="ExternalInput").ap()
    wh1 = nc.dram_tensor("wh1", [128, KC * 128], F16,
                         kind="ExternalInput").ap()
    # bias per pair-half (turns), [128, 2] fp32: col h = b[h*128:(h+1)*128]
    bc = nc.dram_tensor("bc", [128, 2], F32, kind="ExternalInput").ap()
    # block angle bases (turns, mod 1), [128, 2*NB] fp32: col h*NB + i =
    # base at local t = [0, 512, 1024, 1536, 1792][i] for pair-half h
    bs = nc.dram_tensor("bs", [128, 2 * NB], F32, kind="ExternalInput").ap()
    # out row block tb: [128, 4*TB] fp16 (quadrants o1h0|o1h1|o2h0|o2h1)
    outT = nc.dram_tensor("outT", [nt * 128, 4 * TB], F16,
                          kind="ExternalOutput").ap()

    with tile.TileContext(nc) as tc, ExitStack() as ctx:
        const_pool = ctx.enter_context(tc.tile_pool(name="const", bufs=1))
        w_pool = ctx.enter_context(tc.tile_pool(name="w", bufs=1))
        x_pool = ctx.enter_context(tc.tile_pool(name="x", bufs=4))
        psum_pool = ctx.enter_context(tc.tile_pool(name="psum", bufs=3, space="PSUM"))
        psum2_pool = ctx.enter_context(tc.tile_pool(name="psum2", bufs=2, space="PSUM"))
        ang_pool = ctx.enter_context(tc.tile_pool(name="ang", bufs=2, space="PSUM"))
        trig_pool = ctx.enter_context(tc.tile_pool(name="trig", bufs=2))
        rot_pool = ctx.enter_context(tc.tile_pool(name="rot", bufs=2))
        out_pool = ctx.enter_context(tc.tile_pool(name="out", bufs=2))

        # The sync HW ring is the only fast DMA path (~380 GB/s; the scalar
        # ring gets starved to <100 GB/s when sync is active). Stream every
        # large tensor on it in exact need order, x blocks split in halves
        # (separate tiles, so the first matmul group is gated on 1 MB only):
        #   x0a, wh0, x0b, wh1, x1a, x1b, x2a, x2b, x3a, x3b, out0..out3
        half = KC * TB // 2
        x_tiles = []
        for tb in range(nt):
            xa = x_pool.tile([128, half], F16, tag="xa")
            xb = x_pool.tile([128, half], F16, tag="xb")
            x_tiles.append((xa, xb))
        w_sb = []
        for h in range(2):
            whs = w_pool.tile([128, KC * 128], F16, tag=f"wh{h}")
            w_sb.append(whs)
        nc.sync.dma_start(x_tiles[0][0][:], xf[0:128, 0:half])
        nc.sync.dma_start(w_sb[0][:], wh0[:])
        nc.sync.dma_start(x_tiles[0][1][:], xf[0:128, half:2 * half])
        nc.sync.dma_start(w_sb[1][:], wh1[:])
        for tb in range(1, nt):
            nc.sync.dma_start(x_tiles[tb][0][:],
                              xf[tb * 128:(tb + 1) * 128, 0:half])
            nc.sync.dma_start(x_tiles[tb][1][:],
                              xf[tb * 128:(tb + 1) * 128, half:2 * half])

        # tiny consts on the scalar ring (needed by the first scan at ~16us;
        # even the starved ring lands them in time)
        bs_sb = const_pool.tile([128, 2 * NB], F32, tag="bs")
        nc.scalar.dma_start(bs_sb[:], bs[:])
        bc_sb = const_pool.tile([128, 2], F32, tag="bc")
        nc.scalar.dma_start(bc_sb[:], bc[:])

        magic_sb = const_pool.tile([128, 1], F32, tag="magic")
        nc.gpsimd.memset(magic_sb[:], MAGIC)
        cosb_sb = const_pool.tile([128, 1], F32, tag="cosb")
        nc.gpsimd.memset(cosb_sb[:], COS_BIAS)
        # bias broadcast tile for the scan's data1: [128, 2*TB], col block h
        # filled with b[h*128+p] (memset 0 then add the per-partition bias)
        bt_sb = const_pool.tile([128, 2 * TB], F32, tag="bt")
        nc.gpsimd.memset(bt_sb[:], 0.0)
        for h in range(2):
            nc.scalar.activation(bt_sb[:, h * TB:(h + 1) * TB],
                                 bt_sb[:, h * TB:(h + 1) * TB], IDENT,
                                 bias=bc_sb[:, h:h + 1])

        # jobs: (tb, col offset, width, base col idx, tail?). The final 512
        # block is split into two 256 sub-blocks end-to-end (own matmul
        # group, scan, trig, rotation, output DMA) so the post-matmul drain
        # chain at kernel end is half length; its rotation runs all-DVE.
        jobs = [(tb, 0, TB, tb, False) for tb in range(nt - 1)]
        jobs.append((nt - 1, 0, TB // 2, nt - 1, True))
        jobs.append((nt - 1, TB // 2, TB // 2, NB - 1, True))

        for tb, off, w, bi0, tail in jobs:
            xa, xb = x_tiles[tb]
            oall = out_pool.tile([128, 4 * TB // (2 if tail else 1)], F16,
                                 tag="oalls" if tail else "oall")

            sfx = "s" if tail else ""
            for h in range(2):
                # deltas^T in PSUM: single fp16 pass over 16 d-chunks
                if tail:
                    dp = psum2_pool.tile([128, w], F32, tag="dps")
                else:
                    dp = psum_pool.tile([128, w], F32, tag="dp")
                for d in range(KC):
                    ws = slice(d * 128, (d + 1) * 128)
                    xt = xa if d < KC // 2 else xb
                    x0c = (d % (KC // 2)) * TB + off
                    nc.tensor.matmul(dp[:], w_sb[h][:, ws],
                                     xt[:, x0c:x0c + w],
                                     start=(d == 0), stop=(d == KC - 1))

                if tail:
                    ang = trig_pool.tile([128, w], F32, tag="angs")
                else:
                    ang = ang_pool.tile([128, w], F32, tag="ang")
                a_s = trig_pool.tile([128, w], F32, tag="a_s" + sfx)
                rs = trig_pool.tile([128, w], F32, tag="rs" + sfx)
                sin_t = trig_pool.tile([128, w], F16, tag="sin" + sfx)
                ab = trig_pool.tile([128, w], F32, tag="ab" + sfx)
                cos_t = trig_pool.tile([128, w], F16, tag="cos" + sfx)
                t1 = rot_pool.tile([128, w], F16, tag="t1" + sfx)
                t2 = rot_pool.tile([128, w], F16, tag="t2" + sfx)
                t3 = rot_pool.tile([128, w], F16, tag="t3" + sfx)
                t4 = rot_pool.tile([128, w], F16, tag="t4" + sfx)

                # cumulative angle (turns): prefix scan, initial = host base,
                # +b per step via data1
                nc.vector.tensor_tensor_scan(
                    ang[:], dp[:], bt_sb[:, h * TB + off:h * TB + off + w],
                    initial=bs_sb[:, h * NB + bi0:h * NB + bi0 + 1],
                    op0=ADD, op1=ADD)

                # range reduction (turns): rs = y - round(y) in [-.5, .5]
                nc.scalar.activation(a_s[:], ang[:], IDENT,
                                     bias=magic_sb[:], scale=-1.0)
                nc.vector.scalar_tensor_tensor(rs[:], a_s[:], MAGIC, ang[:],
                                               op0=SUB, op1=ADD)
                nc.scalar.activation(sin_t[:], rs[:], SIN, scale=SCALE_2PI)
                # cos(2pi r) = sin(pi/2 - 2pi|r|), |r| <= 0.5
                nc.scalar.activation(ab[:], rs[:], ABS)
                nc.scalar.activation(cos_t[:], ab[:], SIN,
                                     scale=-SCALE_2PI, bias=cosb_sb[:])

                # rotation: x1^T = d-chunk h, x2^T = d-chunk 2+h (in xa)
                x1s = xa[:, h * TB + off:h * TB + off + w]
                x2s = xa[:, (2 + h) * TB + off:(2 + h) * TB + off + w]
                o1 = oall[:, h * w:(h + 1) * w]
                o2 = oall[:, (2 + h) * w:(3 + h) * w]
                nc.vector.tensor_mul(t1[:], x1s, cos_t[:])
                nc.vector.tensor_mul(t2[:], x2s, sin_t[:])
                nc.vector.tensor_sub(o1, t1[:], t2[:])
                g = nc.vector if tail else nc.gpsimd
                nc.vector.tensor_mul(t3[:], x2s, cos_t[:])
                g.tensor_mul(t4[:], x1s, sin_t[:])
                g.tensor_add(o2, t3[:], t4[:])

            # out DMA on the sync queue: all x prefetches are already issued,
            # so its completion wait never blocks them; the ring is idle by
            # the time outputs start flowing.
            ocol = 4 * off  # tail sub-block at off=256 -> out cols 1024:2048
            nc.sync.dma_start(
                outT[tb * 128:(tb + 1) * 128, ocol:ocol + 4 * w], oall[:])

    nc.compile()
    return nc


_NC_CACHE: dict = {}


def _get_nc():
    if "nc" not in _NC_CACHE:
        _NC_CACHE["nc"] = build_program()
    return _NC_CACHE["nc"]


def _tile_x(xt16: np.ndarray, nt: int) -> np.ndarray:
    """[D, tl] fp16 -> [nt*128, KC*TB]: row block tb, d-chunks along free."""
    tl = xt16.shape[1]
    a = xt16.reshape(KC, 128, tl // TB, TB).transpose(2, 1, 0, 3)
    return np.ascontiguousarray(a.reshape((tl // TB) * 128, KC * TB))


def prepare_weights(W: np.ndarray, b: np.ndarray):
    inv2pi = 1.0 / (2.0 * np.pi)
    Wt = W.astype(np.float64).T * inv2pi                           # [D, P]
    bt = b.astype(np.float64) * inv2pi                             # [P]
    whf = Wt.astype(np.float16)
    # [D, P] -> per half h: [128, KC*128] with d-chunks along free dim
    w4 = whf.reshape(KC, 128, 2, 128)                              # d-chunk, dd, h, pp
    wh_ins = [np.ascontiguousarray(
        w4[:, :, h, :].transpose(1, 0, 2).reshape(128, KC * 128))
        for h in range(2)]
    bc_in = np.ascontiguousarray(
        bt.astype(np.float32).reshape(2, 128).T)                   # [128, 2]
    return wh_ins, bc_in, Wt, bt


def make_in_maps(x: np.ndarray, W: np.ndarray, b: np.ndarray):
    B = x.shape[0]
    wh_ins, bc_in, Wt, bt = prepare_weights(W, b)

    # fp64 cumulative angle at every 256-step boundary, per batch (in turns,
    # using the TRUE W so on-device fp16-weight drift spans <= 512 steps):
    # one pass of 256-block sums over x, then a small [16, D] @ [D, P] matmul
    T = x.shape[1]
    SG = 256
    nblk = T // SG                                                  # 16
    xblk = x.reshape(B, nblk, SG, D).sum(axis=2, dtype=np.float64)  # [B,16,D]
    dblk = xblk @ Wt + SG * bt                                      # [B,16,P]
    bases = np.zeros((B, nblk, P))
    np.cumsum(dblk[:, :-1], axis=1, out=bases[:, 1:])               # exclusive
    bases -= np.round(bases)                                        # mod 1

    in_maps = []
    for c in range(N_CORES):
        bb, hh = c // 2, c % 2
        xt16 = x[bb, hh * TL:(hh + 1) * TL, :].T.astype(np.float16)
        # base columns at local t = [0, 512, 1024, 1536, 1792]
        gsel = [hh * 8 + g for g in (0, 2, 4, 6, 7)]
        bs_in = np.empty((128, 2 * NB), np.float32)
        for h in range(2):
            for i, g in enumerate(gsel):
                bs_in[:, h * NB + i] = bases[bb, g, h * 128:(h + 1) * 128]
        in_maps.append({
            "xf": _tile_x(xt16, NT),
            "wh0": wh_ins[0],
            "wh1": wh_ins[1],
            "bc": bc_in,
            "bs": bs_in,
        })
    return in_maps


def assemble_output(x: np.ndarray, results) -> np.ndarray:
    B, T, Din = x.shape
    out = np.empty((B, T, Din), np.float32)
    out[:, :, ROT:] = x[:, :, ROT:]
    for c in range(N_CORES):
        bb, hh = c // 2, c % 2
        rr = results[c]["outT"]
        r = rr[:(NT - 1) * 128].reshape(NT - 1, 128, 4, TB)
        # [tb, pp, q(oi,h), u] -> [t_local(tb,u), p(oi,h,pp)]
        blk = r.transpose(0, 3, 2, 1).reshape(TL - TB, ROT)
        out[bb, hh * TL:hh * TL + TL - TB, :ROT] = blk
        # last block shipped as two 256 sub-blocks: [pp, s, q, u]
        r3 = rr[(NT - 1) * 128:].reshape(128, 2, 4, TB // 2)
        blk3 = r3.transpose(1, 3, 2, 0).reshape(TB, ROT)
        out[bb, hh * TL + TL - TB:(hh + 1) * TL, :ROT] = blk3
    return out


def kernel(x: np.ndarray, W: np.ndarray, b: np.ndarray) -> np.ndarray:
    nc = _get_nc()
    in_maps = make_in_maps(x, W, b)
    res = run_bass_kernel_spmd(nc, in_maps, list(range(N_CORES)))
    return assemble_output(x, res.results)
